# revision 21
# baseline (speedup 1.0000x reference)
"""Trainium2 Bass kernel for CartesianPlaneNonSirenEmbeddingNetwork.

Tri-plane bilinear feature sampling + positional encoding + 3-layer MLP,
data-parallel over 8 NeuronCores (points sharded, planes/weights replicated).

Device strategy (per core, 131072 points):
  - Host packs each plane's used quadrant into a "quad-diff" gather table:
    one 256 B row per grid cell = [D0|D1|D2|D3] x 32 ch (bf16), so that
    bilinear = D0 + wx*D1 + wy*D2 + wx*wy*D3 (one dma_gather per point/plane).
  - GPSIMD dma_gather fetches rows point-major: G[128, ST, 128].
  - DVE does the 3-term interpolation with host-shipped per-point weights
    (broadcast-AP multiplies), writing features point-major f[128, ST, 128].
  - ACT evaluates sin() on host-shipped posenc args (f16) into f.
  - PE transposes f to channel-major and runs the 123->128->128->1 MLP with
    stationary weights; biases fold in via a constant-1 feature row (b1) and
    ACT bias on the relu copy (b2); b3 is added on host.
"""

import os
import numpy as np
import ml_dtypes

import concourse.bass as bass
import concourse.bacc as bacc
import concourse.mybir as mybir
from concourse import library_config
from concourse.bass_utils import run_bass_kernel_spmd

BF16 = ml_dtypes.bfloat16

# Problem shapes (hardcoded).
C, H, W = 32, 256, 256
MULTIRES = 4
B, N = 4, 262144
NPTS = B * N
NCORES = 8
TCORE = NPTS // NCORES          # 131072 points per core

# Tiling.
ST = 32                         # 128-point blocks per tile
TT = 128 * ST                   # 4096 points per tile
NTILES = TCORE // TT            # 32
IDXF = TT // 16                 # 256  (wrapped idx free dim per plane)
NSUB = ST // 4                  # 8    (512-point sub-chunks per tile)

NCELL = 128                     # used cells per axis (coords in [0,1))
NROWS = NCELL * NCELL           # 16384 table rows per plane

PLANE_DIMS = [(0, 1), (1, 2), (0, 2)]   # (u, v) coordinate dims per plane

dt = mybir.dt
Alu = mybir.AluOpType
Act = mybir.ActivationFunctionType


def build_nc():
    nc = bacc.Bacc()

    tabs = [
        nc.declare_dram_parameter(f"tab{p}", [NROWS, 128], dt.bfloat16, False)
        for p in range(3)
    ]
    idx_d = nc.declare_dram_parameter("idx", [NTILES, 128, 3 * IDXF], dt.int16, False)
    wts_d = nc.declare_dram_parameter("wts", [NTILES, 128, ST * 12], dt.bfloat16, False)
    args_d = nc.declare_dram_parameter("args", [NTILES, 128, ST * 24], dt.float16, False)
    xpt_d = nc.declare_dram_parameter("xpt", [NTILES, 128, ST * 3], dt.bfloat16, False)
    w1t_d = nc.declare_dram_parameter("w1t", [128, 128], dt.bfloat16, False)
    w2t_d = nc.declare_dram_parameter("w2t", [128, 128], dt.bfloat16, False)
    w3t_d = nc.declare_dram_parameter("w3t", [128, 1], dt.bfloat16, False)
    b2_d = nc.declare_dram_parameter("b2c", [128, 1], dt.float32, False)
    ident_d = nc.declare_dram_parameter("ident", [128, 128], dt.bfloat16, False)
    y_d = nc.declare_dram_parameter("y", [NTILES, 8, 512], dt.float32, True)

    from contextlib import ExitStack

    with ExitStack() as st:
        e = st.enter_context
        # SBUF
        G_sb = [[e(nc.sbuf_tensor(f"g{s}_{p}", [128, ST * 128], dt.bfloat16))
                 for p in range(3)] for s in range(2)]
        idx_sb = [e(nc.sbuf_tensor(f"idx{s}", [128, 3 * IDXF], dt.int16)) for s in range(2)]
        wts_sb = [e(nc.sbuf_tensor(f"wts{s}", [128, ST * 12], dt.bfloat16)) for s in range(2)]
        args_sb = [e(nc.sbuf_tensor(f"args{s}", [128, ST * 24], dt.float16)) for s in range(2)]
        xpt_sb = [e(nc.sbuf_tensor(f"xpt{s}", [128, ST * 3], dt.bfloat16)) for s in range(2)]
        f_sb = [e(nc.sbuf_tensor(f"f{s}", [128, ST * 128], dt.bfloat16)) for s in range(2)]
        m_sb = [e(nc.sbuf_tensor(f"m{j}", [128, ST * 32], dt.bfloat16))
                for j in range(9)]
        fcm_sb = [e(nc.sbuf_tensor(f"fcm{s}", [128, 512], dt.bfloat16)) for s in range(2)]
        h1_sb = [e(nc.sbuf_tensor(f"h1{s}", [128, 512], dt.bfloat16)) for s in range(2)]
        h2_sb = [e(nc.sbuf_tensor(f"h2{s}", [128, 512], dt.bfloat16)) for s in range(2)]
        y_sb = [e(nc.sbuf_tensor(f"ysb{s}", [128, 512 * (NSUB // 2)], dt.float32)) for s in range(2)]
        w1t_sb = e(nc.sbuf_tensor("w1ts", [128, 128], dt.bfloat16))
        w2t_sb = e(nc.sbuf_tensor("w2ts", [128, 128], dt.bfloat16))
        w3t_sb = e(nc.sbuf_tensor("w3ts", [128, 1], dt.bfloat16))
        b2_sb = e(nc.sbuf_tensor("b2s", [128, 1], dt.float32))
        ident_sb = e(nc.sbuf_tensor("idents", [128, 128], dt.bfloat16))
        # PSUM
        fT_ps = [e(nc.psum_tensor(f"ft{s}", [128, 512], dt.bfloat16)) for s in range(2)]
        v1_ps = [e(nc.psum_tensor(f"v1{s}", [128, 512], dt.float32)) for s in range(2)]
        v2_ps = [e(nc.psum_tensor(f"v2{s}", [128, 512], dt.float32)) for s in range(2)]
        yb_ps = [e(nc.psum_tensor(f"yb{s}", [128, 512], dt.float32)) for s in range(2)]

        with nc.Block() as block:
            sem = lambda n: st.enter_context(nc.semaphore(n))
            init_sem = sem("init_sem")
            g_s = [sem("g0"), sem("g1")]; f_sem = sem("f_sem"); a_sem = sem("a_sem")
            pe_ft = sem("pe_ft"); pe_v1 = sem("pe_v1"); pe_v2 = sem("pe_v2"); pe_yb = sem("pe_yb")
            ac_fcm = sem("ac_fcm"); ac_h1 = sem("ac_h1"); ac_h2 = sem("ac_h2"); ac_y = sem("ac_y")
            ild = [sem("ild0"), sem("ild1")]; wld = [sem("wld0"), sem("wld1")]
            ald = [sem("ald0"), sem("ald1")]; xld = [sem("xld0"), sem("xld1")]
            out_s = [sem("out0"), sem("out1")]
            NINIT = 5 * 16

            @block.sync
            def _(sync):
                sync.dma_start(out=w1t_sb[:, :], in_=w1t_d[:, :]).then_inc(init_sem, 16)
                sync.dma_start(out=w2t_sb[:, :], in_=w2t_d[:, :]).then_inc(init_sem, 16)
                sync.dma_start(out=w3t_sb[:, :], in_=w3t_d[:, :]).then_inc(init_sem, 16)
                sync.dma_start(out=b2_sb[:, :], in_=b2_d[:, :]).then_inc(init_sem, 16)
                sync.dma_start(out=ident_sb[:, :], in_=ident_d[:, :]).then_inc(init_sem, 16)
                for i in range(NTILES):
                    sl = i % 2
                    if i >= 2:
                        # WAR: slot consumers of tile i-2 must be done.
                        sync.wait_ge(g_s[i % 2], 48 * ((i - 2) // 2 + 1))  # idx read
                        sync.wait_ge(f_sem, i - 1)              # wts/xpt read by DVE
                        sync.wait_ge(a_sem, i - 1)              # args read by ACT
                    sync.dma_start(out=idx_sb[sl][:, :], in_=idx_d[i]).then_inc(ild[sl], 16)
                    sync.dma_start(out=wts_sb[sl][:, :], in_=wts_d[i]).then_inc(wld[sl], 16)
                    sync.dma_start(out=args_sb[sl][:, :], in_=args_d[i]).then_inc(ald[sl], 16)
                    sync.dma_start(out=xpt_sb[sl][:, :], in_=xpt_d[i]).then_inc(xld[sl], 16)
                    if i >= 1:
                        io = i - 1
                        for g in range(4):
                            sync.wait_ge(ac_y, 4 * io + g + 1)
                            sync.dma_start(
                                out=y_d[io, 2 * g:2 * (g + 1), :],
                                in_=y_sb[io % 2][0:64:32, 512 * g:512 * (g + 1)],
                            ).then_inc(out_s[io % 2], 16)
                io = NTILES - 1
                for g in range(4):
                    sync.wait_ge(ac_y, 4 * io + g + 1)
                    sync.dma_start(
                        out=y_d[io, 2 * g:2 * (g + 1), :],
                        in_=y_sb[io % 2][0:64:32, 512 * g:512 * (g + 1)],
                    ).then_inc(out_s[io % 2], 16)
                sync.wait_ge(out_s[0], 16 * 4 * ((NTILES + 1) // 2))
                sync.wait_ge(out_s[1], 16 * 4 * (NTILES // 2))

            @block.gpsimd
            def _(gpsimd):
                nogather = os.environ.get("K_NOGATHER") == "1"
                nidx_reg = gpsimd.alloc_register("nidx")
                gpsimd.reg_mov(nidx_reg, TT)
                for i in range(NTILES):
                    sl = i % 2
                    gpsimd.wait_ge(ild[sl], 16 * (i // 2 + 1))   # idx loaded
                    if i >= 2:
                        gpsimd.wait_ge(f_sem, i - 1)             # G slot free
                    for p in range(3):
                        if nogather:
                            gpsimd.dma_start(
                                out=G_sb[sl][p][:, :],
                                in_=tabs[p][0:ST, :].rearrange(
                                    "r v -> (r v)").unsqueeze(0).broadcast_to(
                                    (128, ST * 128)),
                            ).then_inc(g_s[sl], 16)
                            continue
                        gpsimd.dma_gather(
                            G_sb[sl][p][:, :].rearrange("q (s v) -> q s v", v=128),
                            tabs[p][:, :],
                            idx_sb[sl][:, p * IDXF:(p + 1) * IDXF],
                            TT,
                            nidx_reg,
                            128,
                            single_packet=False,
                        ).then_inc(g_s[sl], 16)

            @block.vector
            def _(vector):
                for s in range(2):
                    vector.memset(yb_ps[s][:, :], 0.0)
                vector.drain()
                # init constant feature rows: col 123 = 1.0 (bias row), 124..127 = 0
                for s in range(2):
                    fr = f_sb[s][:, :].rearrange("q (s v) -> q s v", v=128)
                    vector.memset(fr[:, :, 123:124], 1.0)
                    vector.memset(fr[:, :, 124:128], 0.0)
                for i in range(NTILES):
                    sl = i % 2
                    vector.wait_ge(g_s[sl], 48 * (i // 2 + 1))   # all 3 gathers done
                    vector.wait_ge(wld[sl], 16 * (i // 2 + 1))   # wts loaded
                    vector.wait_ge(xld[sl], 16 * (i // 2 + 1))   # xpt loaded
                    if i >= 2:
                        vector.wait_ge(pe_ft, 8 * (i - 1))       # f slot free
                    fr = f_sb[sl][:, :].rearrange("q (s v) -> q s v", v=128)
                    wr = wts_sb[sl][:, :].rearrange("q (s w) -> q s w", w=12)
                    m = [m_sb[j][:, :].rearrange("q (s v) -> q s v", v=32)
                         for j in range(9)]
                    gr = [G_sb[sl][p][:, :].rearrange("q (s v) -> q s v", v=128)
                          for p in range(3)]
                    for p in range(3):
                        for t in range(3):
                            w = wr[:, :, 3 * p + t:3 * p + t + 1].broadcast_to(
                                (128, ST, 32))
                            vector.tensor_tensor(
                                m[3 * p + t], gr[p][:, :, 32 * (t + 1):32 * (t + 2)],
                                w, Alu.mult)
                    vector.drain()
                    for p in range(3):
                        vector.tensor_tensor(m[3 * p], m[3 * p], m[3 * p + 1], Alu.add)
                    vector.drain()
                    for p in range(3):
                        vector.tensor_tensor(m[3 * p], m[3 * p], m[3 * p + 2], Alu.add)
                    vector.drain()
                    for p in range(3):
                        vector.tensor_tensor(
                            fr[:, :, 32 * p:32 * (p + 1)], m[3 * p],
                            gr[p][:, :, 0:32], Alu.add)
                    xr = xpt_sb[sl][:, :].rearrange("q (s v) -> q s v", v=3)
                    vector.drain()
                    vector.tensor_copy(fr[:, :, 96:99], xr).then_inc(f_sem, 1)

            @block.scalar
            def _(scalar):
                for i in range(NTILES):
                    sl = i % 2
                    scalar.wait_ge(ald[sl], 16 * (i // 2 + 1))   # args loaded
                    if i >= 2:
                        scalar.wait_ge(pe_ft, 8 * (i - 1))       # f slot free
                    fr = f_sb[sl][:, :].rearrange("q (s v) -> q s v", v=128)
                    ar = args_sb[sl][:, :].rearrange("q (s v) -> q s v", v=24)
                    scalar.activation(fr[:, :, 99:123], ar, Act.Sin).then_inc(a_sem, 1)
                    for k in range(NSUB + 3):
                        if k < NSUB:
                            kg = 8 * i + k
                            scalar.wait_ge(pe_ft, kg + 1)
                            if kg >= 2:
                                scalar.wait_ge(pe_v1, kg - 1)    # fcm_sb slot free
                            scalar.activation(
                                fcm_sb[k % 2][:, :], fT_ps[k % 2][:, :], Act.Copy
                            ).then_inc(ac_fcm, 1)
                        j = k - 1
                        if 0 <= j < NSUB:
                            jg = 8 * i + j
                            scalar.wait_ge(pe_v1, jg + 1)
                            if jg >= 2:
                                scalar.wait_ge(pe_v2, jg - 1)    # h1_sb slot free
                            scalar.activation(
                                h1_sb[j % 2][:, :], v1_ps[j % 2][:, :], Act.Relu
                            ).then_inc(ac_h1, 1)
                        j = k - 2
                        if 0 <= j < NSUB:
                            jg = 8 * i + j
                            scalar.wait_ge(pe_v2, jg + 1)
                            if jg >= 2:
                                scalar.wait_ge(pe_yb, jg - 1)    # h2_sb slot free
                            scalar.activation(
                                h2_sb[j % 2][:, :], v2_ps[j % 2][:, :], Act.Relu,
                                bias=b2_sb[:, 0:1],
                            ).then_inc(ac_h2, 1)
                        j = k - 3
                        if 0 <= j < NSUB and j % 2 == 1:
                            g = j // 2
                            gg = 4 * i + g
                            scalar.wait_ge(pe_yb, 8 * i + j + 1)
                            if i >= 2 and g == 0:
                                scalar.wait_ge(out_s[i % 2], 16 * 4 * ((i - 2) // 2 + 1))
                            scalar.activation(
                                y_sb[i % 2][0:64, 512 * g:512 * (g + 1)],
                                yb_ps[g % 2][0:64, :],
                                Act.Copy,
                            ).then_inc(ac_y, 1)

            @block.tensor
            def _(tensor):
                tensor.wait_ge(init_sem, NINIT)
                for i in range(NTILES):
                    sl = i % 2
                    fr = f_sb[sl][:, :].rearrange("q (s v) -> q s v", v=128)
                    for k in range(NSUB + 3):
                        if k < NSUB:
                            kg = 8 * i + k
                            if k == 0:
                                tensor.wait_ge(f_sem, i + 1)
                                tensor.wait_ge(a_sem, i + 1)
                            if kg >= 2:
                                tensor.wait_ge(ac_fcm, kg - 1)   # fT bank free
                            for b in range(4):
                                ins = tensor.transpose(
                                    fT_ps[k % 2][:, 128 * b:128 * (b + 1)],
                                    fr[:, 4 * k + b, :],
                                    ident_sb[:, :],
                                )
                            ins.then_inc(pe_ft, 1)
                        j = k - 1
                        if 0 <= j < NSUB:
                            jg = 8 * i + j
                            tensor.wait_ge(ac_fcm, jg + 1)
                            if jg >= 2:
                                tensor.wait_ge(ac_h1, jg - 1)    # v1 bank free
                            tensor.matmul(
                                v1_ps[j % 2][:, :], w1t_sb[:, :], fcm_sb[j % 2][:, :]
                            ).then_inc(pe_v1, 1)
                        j = k - 2
                        if 0 <= j < NSUB:
                            jg = 8 * i + j
                            tensor.wait_ge(ac_h1, jg + 1)
                            if jg >= 2:
                                tensor.wait_ge(ac_h2, jg - 1)    # v2 bank free
                            tensor.matmul(
                                v2_ps[j % 2][:, :], w2t_sb[:, :], h1_sb[j % 2][:, :]
                            ).then_inc(pe_v2, 1)
                        j = k - 3
                        if 0 <= j < NSUB:
                            jg = 8 * i + j
                            g = j // 2
                            gg = 4 * i + g
                            tensor.wait_ge(ac_h2, jg + 1)
                            if gg >= 2 and j % 2 == 0:
                                tensor.wait_ge(ac_y, gg - 1)     # yb bank free
                            tensor.matmul(
                                yb_ps[g % 2][32 * (j % 2):32 * (j % 2) + 1, :],
                                w3t_sb[:, :],
                                h2_sb[j % 2][:, :],
                            ).then_inc(pe_yb, 1)

    nc.compile()
    return nc


def _host_prep(coordinates, plane0, plane1, plane2, W1, b1, W2, b2, W3, b3):
    """Build all device inputs. Returns (shared, per_core_list, b3)."""
    f32 = np.float32
    pts = np.ascontiguousarray(coordinates.reshape(NPTS, 3).astype(f32))

    # --- tables ----------------------------------------------------------
    tabs = []
    for pl in (plane0, plane1, plane2):
        q = np.asarray(pl, dtype=f32)[:, 127:256, 127:256]      # [32,129,129]
        g00 = q[:, :128, :128]
        g01 = q[:, :128, 1:129]
        g10 = q[:, 1:129, :128]
        g11 = q[:, 1:129, 1:129]
        d = np.stack([g00, g01 - g00, g10 - g00, g11 - g01 - g10 + g00], axis=0)
        # [term, c, ly, lx] -> [ly, lx, term, c] -> [16384, 128]
        tab = np.transpose(d, (2, 3, 0, 1)).reshape(NROWS, 128).astype(BF16)
        tabs.append(np.ascontiguousarray(tab))

    # --- per-point quantities -------------------------------------------
    fx = (pts + f32(1.0)) * f32(0.5) * f32(255.0)               # [NPTS,3]
    x0 = np.floor(fx)
    fr = (fx - x0).astype(f32)                                  # fractional parts
    cell = (x0.astype(np.int32) - 127)                          # [NPTS,3] in [0,127]

    idx_all = np.empty((NPTS, 3), np.int16)
    wts_all = np.zeros((NPTS, 12), f32)
    for p, (ua, va) in enumerate(PLANE_DIMS):
        idx_all[:, p] = (cell[:, va] * NCELL + cell[:, ua]).astype(np.int16)
        wts_all[:, 3 * p + 0] = fr[:, ua]
        wts_all[:, 3 * p + 1] = fr[:, va]
        wts_all[:, 3 * p + 2] = fr[:, ua] * fr[:, va]
    wts_all = wts_all.astype(BF16)

    freqs = (2.0 ** np.linspace(0.0, MULTIRES - 1.0, MULTIRES)).astype(f32)
    args_all = np.empty((NPTS, 24), f32)
    for i, f in enumerate(freqs):
        args_all[:, 6 * i:6 * i + 3] = pts * f
        args_all[:, 6 * i + 3:6 * i + 6] = pts * f + f32(np.pi / 2)
    # ACT Sin domain is [-pi, pi]: exact periodic range reduction (float64).
    a64 = args_all.astype(np.float64)
    a64 = a64 - 2 * np.pi * np.round(a64 / (2 * np.pi))
    args_all = np.clip(a64, -np.pi, np.pi).astype(np.float16)
    xpt_all = pts.astype(BF16)

    # --- weights ---------------------------------------------------------
    w1t = np.zeros((128, 128), f32)
    w1t[:123, :] = np.asarray(W1, f32).T                        # [123,128]
    w1t[123, :] = np.asarray(b1, f32)
    w2t = np.asarray(W2, f32).T.astype(BF16)                    # [128,128]
    w3t = np.asarray(W3, f32).T.astype(BF16)                    # [128,1]
    b2c = np.ascontiguousarray(np.asarray(b2, f32).reshape(128, 1))
    ident = np.eye(128, dtype=BF16)
    shared = dict(
        tab0=tabs[0], tab1=tabs[1], tab2=tabs[2],
        w1t=w1t.astype(BF16), w2t=w2t, w3t=w3t, b2c=b2c, ident=ident,
    )

    def tile_pm(a, core):
        """[TCORE, M] slice of a per-point array -> [NTILES, 128, ST*M],
        point j=128*s+p of tile i at [i, p, s, :]."""
        m = a.shape[1]
        v = a[core * TCORE:(core + 1) * TCORE].reshape(NTILES, ST, 128, m)
        return np.ascontiguousarray(
            v.transpose(0, 2, 1, 3).reshape(NTILES, 128, ST * m)
        )

    per_core = []
    for core in range(NCORES):
        idx_c = idx_all[core * TCORE:(core + 1) * TCORE]        # [TCORE,3]
        # wrapped layout per tile/plane: [16, IDXF], idx j at [j%16, j//16],
        # then replicated x8 down partitions.
        iv = idx_c.reshape(NTILES, TT, 3).transpose(0, 2, 1)    # [NT,3,TT]
        iw = iv.reshape(NTILES, 3, IDXF, 16).transpose(0, 1, 3, 2)  # [NT,3,16,IDXF]
        iw = np.broadcast_to(iw[:, None], (NTILES, 8, 3, 16, IDXF))
        iw = iw.transpose(0, 2, 1, 3, 4).reshape(NTILES, 3, 128, IDXF)
        iw = iw.transpose(0, 2, 1, 3).reshape(NTILES, 128, 3 * IDXF)
        per_core.append(dict(
            idx=np.ascontiguousarray(iw),
            wts=tile_pm(wts_all, core),
            args=tile_pm(args_all, core),
            xpt=tile_pm(xpt_all, core),
        ))
    return shared, per_core


_NC_CACHE = {}


def _get_nc():
    if "nc" not in _NC_CACHE:
        _NC_CACHE["nc"] = build_nc()
    return _NC_CACHE["nc"]


def kernel(coordinates, plane0, plane1, plane2, W1, b1, W2, b2, W3, b3):
    shared, per_core, = _host_prep(
        coordinates, plane0, plane1, plane2, W1, b1, W2, b2, W3, b3
    )[:2]
    nc = _get_nc()
    in_maps = [{**shared, **per_core[c]} for c in range(NCORES)]
    res = run_bass_kernel_spmd(nc, in_maps, list(range(NCORES)))
    ys = [np.asarray(res.results[c]["y"], np.float32).reshape(TCORE)
          for c in range(NCORES)]
    y = np.concatenate(ys) + np.float32(np.asarray(b3, np.float32).reshape(()))
    return y.reshape(B, N, 1).astype(np.float32)


# revision 24
# speedup vs baseline: 1.1609x; 1.1609x over previous
"""Trainium2 Bass kernel for CartesianPlaneNonSirenEmbeddingNetwork.

Tri-plane bilinear feature sampling + positional encoding + 3-layer MLP,
data-parallel over 8 NeuronCores (points sharded, planes/weights replicated).

Device strategy (per core, 131072 points):
  - Host packs each plane's used quadrant into a "quad-diff" gather table:
    one 256 B row per grid cell = [D0|D1|D2|D3] x 32 ch (bf16), so that
    bilinear = D0 + wx*D1 + wy*D2 + wx*wy*D3 (one dma_gather per point/plane).
  - GPSIMD dma_gather fetches rows point-major: G[128, ST, 128].
  - DVE does the 3-term interpolation with host-shipped per-point weights
    (broadcast-AP multiplies), writing features point-major f[128, ST, 128].
  - ACT evaluates sin() on host-shipped posenc args (f16) into f.
  - PE transposes f to channel-major and runs the 123->128->128->1 MLP with
    stationary weights; biases fold in via a constant-1 feature row (b1) and
    ACT bias on the relu copy (b2); b3 is added on host.
"""

import os
import numpy as np
import ml_dtypes

import concourse.bass as bass
import concourse.bacc as bacc
import concourse.mybir as mybir
from concourse import library_config
from concourse.bass_utils import run_bass_kernel_spmd

BF16 = ml_dtypes.bfloat16

# Problem shapes (hardcoded).
C, H, W = 32, 256, 256
MULTIRES = 4
B, N = 4, 262144
NPTS = B * N
NCORES = 8
TCORE = NPTS // NCORES          # 131072 points per core

# Tiling.
ST = 32                         # 128-point blocks per tile
TT = 128 * ST                   # 4096 points per tile
NTILES = TCORE // TT            # 32
IDXF = TT // 16                 # 256  (wrapped idx free dim per plane)
NSUB = ST // 4                  # 8    (512-point sub-chunks per tile)

NCELL = 128                     # used cells per axis (coords in [0,1))
NROWS = NCELL * NCELL           # 16384 table rows per plane

PLANE_DIMS = [(0, 1), (1, 2), (0, 2)]   # (u, v) coordinate dims per plane

dt = mybir.dt
Alu = mybir.AluOpType
Act = mybir.ActivationFunctionType


def build_nc():
    nc = bacc.Bacc()

    tabs = [
        nc.declare_dram_parameter(f"tab{p}", [NROWS, 128], dt.bfloat16, False)
        for p in range(3)
    ]
    idx_d = nc.declare_dram_parameter("idx", [NTILES, 128, 3 * IDXF], dt.int16, False)
    wts_d = nc.declare_dram_parameter("wts", [NTILES, 128, ST * 12], dt.bfloat16, False)
    args_d = nc.declare_dram_parameter("args", [NTILES, 128, ST * 24], dt.float16, False)
    xpt_d = nc.declare_dram_parameter("xpt", [NTILES, 128, ST * 3], dt.bfloat16, False)
    w1t_d = nc.declare_dram_parameter("w1t", [128, 128], dt.bfloat16, False)
    w2t_d = nc.declare_dram_parameter("w2t", [128, 128], dt.bfloat16, False)
    w3t_d = nc.declare_dram_parameter("w3t", [128, 1], dt.bfloat16, False)
    b2_d = nc.declare_dram_parameter("b2c", [128, 1], dt.float32, False)
    ident_d = nc.declare_dram_parameter("ident", [128, 128], dt.bfloat16, False)
    y_d = nc.declare_dram_parameter("y", [NTILES, 8, 512], dt.float32, True)

    from contextlib import ExitStack

    with ExitStack() as st:
        e = st.enter_context
        # SBUF
        G_sb = [[e(nc.sbuf_tensor(f"g{s}_{p}", [128, ST * 128], dt.bfloat16))
                 for p in range(3)] for s in range(2)]
        idx_sb = [e(nc.sbuf_tensor(f"idx{s}", [128, 3 * IDXF], dt.int16)) for s in range(2)]
        wts_sb = [e(nc.sbuf_tensor(f"wts{s}", [128, ST * 12], dt.bfloat16)) for s in range(2)]
        args_sb = [e(nc.sbuf_tensor(f"args{s}", [128, ST * 24], dt.float16)) for s in range(2)]
        xpt_sb = [e(nc.sbuf_tensor(f"xpt{s}", [128, ST * 3], dt.bfloat16)) for s in range(2)]
        f_sb = [e(nc.sbuf_tensor(f"f{s}", [128, ST * 128], dt.bfloat16)) for s in range(2)]
        m_sb = [e(nc.sbuf_tensor(f"m{j}", [128, ST * 32], dt.bfloat16))
                for j in range(9)]
        fcm_sb = [e(nc.sbuf_tensor(f"fcm{s}", [128, 512], dt.bfloat16)) for s in range(2)]
        h1_sb = [e(nc.sbuf_tensor(f"h1{s}", [128, 512], dt.bfloat16)) for s in range(2)]
        h2_sb = [e(nc.sbuf_tensor(f"h2{s}", [128, 512], dt.bfloat16)) for s in range(2)]
        y_sb = [e(nc.sbuf_tensor(f"ysb{s}", [128, 512 * (NSUB // 2)], dt.float32)) for s in range(2)]
        w1t_sb = e(nc.sbuf_tensor("w1ts", [128, 128], dt.bfloat16))
        w2t_sb = e(nc.sbuf_tensor("w2ts", [128, 128], dt.bfloat16))
        w3t_sb = e(nc.sbuf_tensor("w3ts", [128, 1], dt.bfloat16))
        b2_sb = e(nc.sbuf_tensor("b2s", [128, 1], dt.float32))
        ident_sb = e(nc.sbuf_tensor("idents", [128, 128], dt.bfloat16))
        # PSUM
        fT_ps = [e(nc.psum_tensor(f"ft{s}", [128, 512], dt.bfloat16)) for s in range(2)]
        v1_ps = [e(nc.psum_tensor(f"v1{s}", [128, 512], dt.float32)) for s in range(2)]
        v2_ps = [e(nc.psum_tensor(f"v2{s}", [128, 512], dt.float32)) for s in range(2)]
        yb_ps = [e(nc.psum_tensor(f"yb{s}", [128, 512], dt.float32)) for s in range(2)]

        with nc.Block() as block:
            sem = lambda n: st.enter_context(nc.semaphore(n))
            init_sem = sem("init_sem")
            g_s = [[sem(f"g{s}_{p}") for p in range(3)] for s in range(2)]
            f_sem = sem("f_sem"); a_sem = sem("a_sem")
            pe_ft = sem("pe_ft"); pe_v1 = sem("pe_v1"); pe_v2 = sem("pe_v2"); pe_yb = sem("pe_yb")
            ac_fcm = sem("ac_fcm"); ac_h1 = sem("ac_h1"); ac_h2 = sem("ac_h2"); ac_y = sem("ac_y")
            ild = [sem("ild0"), sem("ild1")]; wld = [sem("wld0"), sem("wld1")]
            ald = [sem("ald0"), sem("ald1")]; xld = [sem("xld0"), sem("xld1")]
            out_s = [sem("out0"), sem("out1")]
            NINIT = 5 * 16

            @block.sync
            def _(sync):
                sync.dma_start(out=w1t_sb[:, :], in_=w1t_d[:, :]).then_inc(init_sem, 16)
                sync.dma_start(out=w2t_sb[:, :], in_=w2t_d[:, :]).then_inc(init_sem, 16)
                sync.dma_start(out=w3t_sb[:, :], in_=w3t_d[:, :]).then_inc(init_sem, 16)
                sync.dma_start(out=b2_sb[:, :], in_=b2_d[:, :]).then_inc(init_sem, 16)
                sync.dma_start(out=ident_sb[:, :], in_=ident_d[:, :]).then_inc(init_sem, 16)
                for i in range(NTILES):
                    sl = i % 2
                    if i >= 2:
                        # WAR: slot consumers of tile i-2 must be done.
                        for p in range(3):   # idx read by gathers of tile i-2
                            sync.wait_ge(g_s[i % 2][p], 16 * ((i - 2) // 2 + 1))
                        sync.wait_ge(f_sem, i - 1)              # wts/xpt read by DVE
                        sync.wait_ge(a_sem, i - 1)              # args read by ACT
                    sync.dma_start(out=idx_sb[sl][:, :], in_=idx_d[i]).then_inc(ild[sl], 16)
                    sync.dma_start(out=wts_sb[sl][:, :], in_=wts_d[i]).then_inc(wld[sl], 16)
                    sync.dma_start(out=args_sb[sl][:, :], in_=args_d[i]).then_inc(ald[sl], 16)
                    sync.dma_start(out=xpt_sb[sl][:, :], in_=xpt_d[i]).then_inc(xld[sl], 16)
                    if i >= 1:
                        io = i - 1
                        for g in range(4):
                            sync.wait_ge(ac_y, 4 * io + g + 1)
                            sync.dma_start(
                                out=y_d[io, 2 * g:2 * (g + 1), :],
                                in_=y_sb[io % 2][0:64:32, 512 * g:512 * (g + 1)],
                            ).then_inc(out_s[io % 2], 16)
                io = NTILES - 1
                for g in range(4):
                    sync.wait_ge(ac_y, 4 * io + g + 1)
                    sync.dma_start(
                        out=y_d[io, 2 * g:2 * (g + 1), :],
                        in_=y_sb[io % 2][0:64:32, 512 * g:512 * (g + 1)],
                    ).then_inc(out_s[io % 2], 16)
                sync.wait_ge(out_s[0], 16 * 4 * ((NTILES + 1) // 2))
                sync.wait_ge(out_s[1], 16 * 4 * (NTILES // 2))

            @block.gpsimd
            def _(gpsimd):
                nogather = os.environ.get("K_NOGATHER") == "1"
                nidx_reg = gpsimd.alloc_register("nidx")
                gpsimd.reg_mov(nidx_reg, TT)
                for i in range(NTILES):
                    sl = i % 2
                    gpsimd.wait_ge(ild[sl], 16 * (i // 2 + 1))   # idx loaded
                    if i >= 2:
                        gpsimd.wait_ge(f_sem, i - 1)             # G slot free
                    for p in range(3):
                        if nogather:
                            gpsimd.dma_start(
                                out=G_sb[sl][p][:, :],
                                in_=tabs[p][0:ST, :].rearrange(
                                    "r v -> (r v)").unsqueeze(0).broadcast_to(
                                    (128, ST * 128)),
                            ).then_inc(g_s[sl][p], 16)
                            continue
                        gpsimd.dma_gather(
                            G_sb[sl][p][:, :].rearrange("q (s v) -> q s v", v=128),
                            tabs[p][:, :],
                            idx_sb[sl][:, p * IDXF:(p + 1) * IDXF],
                            TT,
                            nidx_reg,
                            128,
                            single_packet=False,
                        ).then_inc(g_s[sl][p], 16)

            @block.vector
            def _(vector):
                for s in range(2):
                    vector.memset(yb_ps[s][:, :], 0.0)
                vector.drain()
                # init constant feature rows: col 123 = 1.0 (bias row), 124..127 = 0
                for s in range(2):
                    fr = f_sb[s][:, :].rearrange("q (s v) -> q s v", v=128)
                    vector.memset(fr[:, :, 123:124], 1.0)
                    vector.memset(fr[:, :, 124:128], 0.0)
                for i in range(NTILES):
                    sl = i % 2
                    vector.wait_ge(wld[sl], 16 * (i // 2 + 1))   # wts loaded
                    vector.wait_ge(xld[sl], 16 * (i // 2 + 1))   # xpt loaded
                    if i >= 2:
                        vector.wait_ge(pe_ft, 8 * (i - 1))       # f slot free
                    fr = f_sb[sl][:, :].rearrange("q (s v) -> q s v", v=128)
                    wr = wts_sb[sl][:, :].rearrange("q (s w) -> q s w", w=12)
                    m = [m_sb[j][:, :].rearrange("q (s v) -> q s v", v=32)
                         for j in range(9)]
                    gr = [G_sb[sl][p][:, :].rearrange("q (s v) -> q s v", v=128)
                          for p in range(3)]
                    for p in range(3):
                        # start as soon as THIS plane's gather has landed
                        vector.wait_ge(g_s[sl][p], 16 * (i // 2 + 1))
                        for t in range(3):
                            w = wr[:, :, 3 * p + t:3 * p + t + 1].broadcast_to(
                                (128, ST, 32))
                            vector.tensor_tensor(
                                m[3 * p + t], gr[p][:, :, 32 * (t + 1):32 * (t + 2)],
                                w, Alu.mult)
                    vector.drain()
                    for p in range(3):
                        vector.tensor_tensor(m[3 * p], m[3 * p], m[3 * p + 1], Alu.add)
                    vector.drain()
                    for p in range(3):
                        vector.tensor_tensor(m[3 * p], m[3 * p], m[3 * p + 2], Alu.add)
                    vector.drain()
                    for p in range(3):
                        vector.tensor_tensor(
                            fr[:, :, 32 * p:32 * (p + 1)], m[3 * p],
                            gr[p][:, :, 0:32], Alu.add)
                    xr = xpt_sb[sl][:, :].rearrange("q (s v) -> q s v", v=3)
                    vector.drain()
                    vector.tensor_copy(fr[:, :, 96:99], xr).then_inc(f_sem, 1)

            @block.scalar
            def _(scalar):
                for i in range(NTILES):
                    sl = i % 2
                    scalar.wait_ge(ald[sl], 16 * (i // 2 + 1))   # args loaded
                    if i >= 2:
                        scalar.wait_ge(pe_ft, 8 * (i - 1))       # f slot free
                    fr = f_sb[sl][:, :].rearrange("q (s v) -> q s v", v=128)
                    ar = args_sb[sl][:, :].rearrange("q (s v) -> q s v", v=24)
                    scalar.activation(fr[:, :, 99:123], ar, Act.Sin).then_inc(a_sem, 1)
                    for k in range(NSUB + 3):
                        if k < NSUB:
                            kg = 8 * i + k
                            scalar.wait_ge(pe_ft, kg + 1)
                            if kg >= 2:
                                scalar.wait_ge(pe_v1, kg - 1)    # fcm_sb slot free
                            scalar.activation(
                                fcm_sb[k % 2][:, :], fT_ps[k % 2][:, :], Act.Copy
                            ).then_inc(ac_fcm, 1)
                        j = k - 1
                        if 0 <= j < NSUB:
                            jg = 8 * i + j
                            scalar.wait_ge(pe_v1, jg + 1)
                            if jg >= 2:
                                scalar.wait_ge(pe_v2, jg - 1)    # h1_sb slot free
                            scalar.activation(
                                h1_sb[j % 2][:, :], v1_ps[j % 2][:, :], Act.Relu
                            ).then_inc(ac_h1, 1)
                        j = k - 2
                        if 0 <= j < NSUB:
                            jg = 8 * i + j
                            scalar.wait_ge(pe_v2, jg + 1)
                            if jg >= 2:
                                scalar.wait_ge(pe_yb, jg - 1)    # h2_sb slot free
                            scalar.activation(
                                h2_sb[j % 2][:, :], v2_ps[j % 2][:, :], Act.Relu,
                                bias=b2_sb[:, 0:1],
                            ).then_inc(ac_h2, 1)
                        j = k - 3
                        if 0 <= j < NSUB and j % 2 == 1:
                            g = j // 2
                            gg = 4 * i + g
                            scalar.wait_ge(pe_yb, 8 * i + j + 1)
                            if i >= 2 and g == 0:
                                scalar.wait_ge(out_s[i % 2], 16 * 4 * ((i - 2) // 2 + 1))
                            scalar.activation(
                                y_sb[i % 2][0:64, 512 * g:512 * (g + 1)],
                                yb_ps[g % 2][0:64, :],
                                Act.Copy,
                            ).then_inc(ac_y, 1)

            @block.tensor
            def _(tensor):
                tensor.wait_ge(init_sem, NINIT)
                for i in range(NTILES):
                    sl = i % 2
                    fr = f_sb[sl][:, :].rearrange("q (s v) -> q s v", v=128)
                    for k in range(NSUB + 3):
                        if k < NSUB:
                            kg = 8 * i + k
                            if k == 0:
                                tensor.wait_ge(f_sem, i + 1)
                                tensor.wait_ge(a_sem, i + 1)
                            if kg >= 2:
                                tensor.wait_ge(ac_fcm, kg - 1)   # fT bank free
                            for b in range(4):
                                ins = tensor.transpose(
                                    fT_ps[k % 2][:, 128 * b:128 * (b + 1)],
                                    fr[:, 4 * k + b, :],
                                    ident_sb[:, :],
                                )
                            ins.then_inc(pe_ft, 1)
                        j = k - 1
                        if 0 <= j < NSUB:
                            jg = 8 * i + j
                            tensor.wait_ge(ac_fcm, jg + 1)
                            if jg >= 2:
                                tensor.wait_ge(ac_h1, jg - 1)    # v1 bank free
                            tensor.matmul(
                                v1_ps[j % 2][:, :], w1t_sb[:, :], fcm_sb[j % 2][:, :]
                            ).then_inc(pe_v1, 1)
                        j = k - 2
                        if 0 <= j < NSUB:
                            jg = 8 * i + j
                            tensor.wait_ge(ac_h1, jg + 1)
                            if jg >= 2:
                                tensor.wait_ge(ac_h2, jg - 1)    # v2 bank free
                            tensor.matmul(
                                v2_ps[j % 2][:, :], w2t_sb[:, :], h1_sb[j % 2][:, :]
                            ).then_inc(pe_v2, 1)
                        j = k - 3
                        if 0 <= j < NSUB:
                            jg = 8 * i + j
                            g = j // 2
                            gg = 4 * i + g
                            tensor.wait_ge(ac_h2, jg + 1)
                            if gg >= 2 and j % 2 == 0:
                                tensor.wait_ge(ac_y, gg - 1)     # yb bank free
                            tensor.matmul(
                                yb_ps[g % 2][32 * (j % 2):32 * (j % 2) + 1, :],
                                w3t_sb[:, :],
                                h2_sb[j % 2][:, :],
                            ).then_inc(pe_yb, 1)

    nc.compile()
    return nc


def _host_prep(coordinates, plane0, plane1, plane2, W1, b1, W2, b2, W3, b3):
    """Build all device inputs. Returns (shared, per_core_list, b3)."""
    f32 = np.float32
    pts = np.ascontiguousarray(coordinates.reshape(NPTS, 3).astype(f32))

    # --- tables ----------------------------------------------------------
    tabs = []
    for pl in (plane0, plane1, plane2):
        q = np.asarray(pl, dtype=f32)[:, 127:256, 127:256]      # [32,129,129]
        g00 = q[:, :128, :128]
        g01 = q[:, :128, 1:129]
        g10 = q[:, 1:129, :128]
        g11 = q[:, 1:129, 1:129]
        d = np.stack([g00, g01 - g00, g10 - g00, g11 - g01 - g10 + g00], axis=0)
        # [term, c, ly, lx] -> [ly, lx, term, c] -> [16384, 128]
        tab = np.transpose(d, (2, 3, 0, 1)).reshape(NROWS, 128).astype(BF16)
        tabs.append(np.ascontiguousarray(tab))

    # --- per-point quantities -------------------------------------------
    fx = (pts + f32(1.0)) * f32(0.5) * f32(255.0)               # [NPTS,3]
    x0 = np.floor(fx)
    fr = (fx - x0).astype(f32)                                  # fractional parts
    cell = (x0.astype(np.int32) - 127)                          # [NPTS,3] in [0,127]

    idx_all = np.empty((NPTS, 3), np.int16)
    wts_all = np.zeros((NPTS, 12), f32)
    for p, (ua, va) in enumerate(PLANE_DIMS):
        idx_all[:, p] = (cell[:, va] * NCELL + cell[:, ua]).astype(np.int16)
        wts_all[:, 3 * p + 0] = fr[:, ua]
        wts_all[:, 3 * p + 1] = fr[:, va]
        wts_all[:, 3 * p + 2] = fr[:, ua] * fr[:, va]
    wts_all = wts_all.astype(BF16)

    freqs = (2.0 ** np.linspace(0.0, MULTIRES - 1.0, MULTIRES)).astype(f32)
    args_all = np.empty((NPTS, 24), f32)
    for i, f in enumerate(freqs):
        args_all[:, 6 * i:6 * i + 3] = pts * f
        args_all[:, 6 * i + 3:6 * i + 6] = pts * f + f32(np.pi / 2)
    # ACT Sin domain is [-pi, pi]: exact periodic range reduction (float64).
    a64 = args_all.astype(np.float64)
    a64 = a64 - 2 * np.pi * np.round(a64 / (2 * np.pi))
    args_all = np.clip(a64, -np.pi, np.pi).astype(np.float16)
    xpt_all = pts.astype(BF16)

    # --- weights ---------------------------------------------------------
    w1t = np.zeros((128, 128), f32)
    w1t[:123, :] = np.asarray(W1, f32).T                        # [123,128]
    w1t[123, :] = np.asarray(b1, f32)
    w2t = np.asarray(W2, f32).T.astype(BF16)                    # [128,128]
    w3t = np.asarray(W3, f32).T.astype(BF16)                    # [128,1]
    b2c = np.ascontiguousarray(np.asarray(b2, f32).reshape(128, 1))
    ident = np.eye(128, dtype=BF16)
    shared = dict(
        tab0=tabs[0], tab1=tabs[1], tab2=tabs[2],
        w1t=w1t.astype(BF16), w2t=w2t, w3t=w3t, b2c=b2c, ident=ident,
    )

    def tile_pm(a, core):
        """[TCORE, M] slice of a per-point array -> [NTILES, 128, ST*M],
        point j=128*s+p of tile i at [i, p, s, :]."""
        m = a.shape[1]
        v = a[core * TCORE:(core + 1) * TCORE].reshape(NTILES, ST, 128, m)
        return np.ascontiguousarray(
            v.transpose(0, 2, 1, 3).reshape(NTILES, 128, ST * m)
        )

    per_core = []
    for core in range(NCORES):
        idx_c = idx_all[core * TCORE:(core + 1) * TCORE]        # [TCORE,3]
        # wrapped layout per tile/plane: [16, IDXF], idx j at [j%16, j//16],
        # then replicated x8 down partitions.
        iv = idx_c.reshape(NTILES, TT, 3).transpose(0, 2, 1)    # [NT,3,TT]
        iw = iv.reshape(NTILES, 3, IDXF, 16).transpose(0, 1, 3, 2)  # [NT,3,16,IDXF]
        iw = np.broadcast_to(iw[:, None], (NTILES, 8, 3, 16, IDXF))
        iw = iw.transpose(0, 2, 1, 3, 4).reshape(NTILES, 3, 128, IDXF)
        iw = iw.transpose(0, 2, 1, 3).reshape(NTILES, 128, 3 * IDXF)
        per_core.append(dict(
            idx=np.ascontiguousarray(iw),
            wts=tile_pm(wts_all, core),
            args=tile_pm(args_all, core),
            xpt=tile_pm(xpt_all, core),
        ))
    return shared, per_core


_NC_CACHE = {}


def _get_nc():
    if "nc" not in _NC_CACHE:
        _NC_CACHE["nc"] = build_nc()
    return _NC_CACHE["nc"]


def kernel(coordinates, plane0, plane1, plane2, W1, b1, W2, b2, W3, b3):
    args = [np.asarray(a) for a in
            (coordinates, plane0, plane1, plane2, W1, b1, W2, b2, W3, b3)]
    shared, per_core = _host_prep(*args)[:2]
    b3 = args[-1]
    nc = _get_nc()
    in_maps = [{**shared, **per_core[c]} for c in range(NCORES)]
    res = run_bass_kernel_spmd(nc, in_maps, list(range(NCORES)))
    ys = [np.asarray(res.results[c]["y"], np.float32).reshape(TCORE)
          for c in range(NCORES)]
    y = np.concatenate(ys) + np.float32(np.asarray(b3, np.float32).reshape(()))
    return y.reshape(B, N, 1).astype(np.float32)


# revision 25
# speedup vs baseline: 1.2547x; 1.0808x over previous
"""Trainium2 Bass kernel for CartesianPlaneNonSirenEmbeddingNetwork.

Tri-plane bilinear feature sampling + positional encoding + 3-layer MLP,
data-parallel over 8 NeuronCores (points sharded, planes/weights replicated).

Device strategy (per core, 131072 points):
  - Host packs each plane's used quadrant into a "quad-diff" gather table:
    one 256 B row per grid cell = [D0|D1|D2|D3] x 32 ch (bf16), so that
    bilinear = D0 + wx*D1 + wy*D2 + wx*wy*D3 (one dma_gather per point/plane).
  - GPSIMD dma_gather fetches rows point-major: G[128, ST, 128].
  - DVE does the 3-term interpolation with host-shipped per-point weights
    (broadcast-AP multiplies), writing features point-major f[128, ST, 128].
  - ACT evaluates sin() on host-shipped posenc args (f16) into f.
  - PE transposes f to channel-major and runs the 123->128->128->1 MLP with
    stationary weights; biases fold in via a constant-1 feature row (b1) and
    ACT bias on the relu copy (b2); b3 is added on host.
"""

import os
import numpy as np
import ml_dtypes

import concourse.bass as bass
import concourse.bacc as bacc
import concourse.mybir as mybir
from concourse import library_config
from concourse.bass_utils import run_bass_kernel_spmd

BF16 = ml_dtypes.bfloat16

# Problem shapes (hardcoded).
C, H, W = 32, 256, 256
MULTIRES = 4
B, N = 4, 262144
NPTS = B * N
NCORES = 8
TCORE = NPTS // NCORES          # 131072 points per core

# Tiling.
ST = 32                         # 128-point blocks per tile
TT = 128 * ST                   # 4096 points per tile
NTILES = TCORE // TT            # 32
IDXF = TT // 16                 # 256  (wrapped idx free dim per plane)
NSUB = ST // 4                  # 8    (512-point sub-chunks per tile)

NCELL = 128                     # used cells per axis (coords in [0,1))
NROWS = NCELL * NCELL           # 16384 table rows per plane

PLANE_DIMS = [(0, 1), (1, 2), (0, 2)]   # (u, v) coordinate dims per plane

dt = mybir.dt
Alu = mybir.AluOpType
Act = mybir.ActivationFunctionType


def build_nc():
    nc = bacc.Bacc()

    tabs = [
        nc.declare_dram_parameter(f"tab{p}", [NROWS, 128], dt.bfloat16, False)
        for p in range(3)
    ]
    idx_d = nc.declare_dram_parameter("idx", [NTILES, 128, 3 * IDXF], dt.int16, False)
    wts_d = nc.declare_dram_parameter("wts", [NTILES, 128, ST * 12], dt.bfloat16, False)
    args_d = nc.declare_dram_parameter("args", [NTILES, 128, ST * 24], dt.float16, False)
    xpt_d = nc.declare_dram_parameter("xpt", [NTILES, 128, ST * 3], dt.bfloat16, False)
    w1t_d = nc.declare_dram_parameter("w1t", [128, 128], dt.bfloat16, False)
    w2t_d = nc.declare_dram_parameter("w2t", [128, 128], dt.bfloat16, False)
    w3t_d = nc.declare_dram_parameter("w3t", [128, 1], dt.bfloat16, False)
    b2_d = nc.declare_dram_parameter("b2c", [128, 1], dt.float32, False)
    ident_d = nc.declare_dram_parameter("ident", [128, 128], dt.bfloat16, False)
    y_d = nc.declare_dram_parameter("y", [NTILES, 8, 512], dt.float32, True)

    from contextlib import ExitStack

    with ExitStack() as st:
        e = st.enter_context
        # SBUF
        G_sb = [[e(nc.sbuf_tensor(f"g{s}_{p}", [128, ST * 128], dt.bfloat16))
                 for p in range(3)] for s in range(2)]
        idx_sb = [e(nc.sbuf_tensor(f"idx{s}", [128, 3 * IDXF], dt.int16)) for s in range(2)]
        wts_sb = [e(nc.sbuf_tensor(f"wts{s}", [128, ST * 12], dt.bfloat16)) for s in range(2)]
        args_sb = [e(nc.sbuf_tensor(f"args{s}", [128, ST * 24], dt.float16)) for s in range(2)]
        xpt_sb = [e(nc.sbuf_tensor(f"xpt{s}", [128, ST * 3], dt.bfloat16)) for s in range(2)]
        f_sb = [e(nc.sbuf_tensor(f"f{s}", [128, ST * 128], dt.bfloat16)) for s in range(2)]
        m_sb = [e(nc.sbuf_tensor(f"m{j}", [128, ST * 32], dt.bfloat16))
                for j in range(9)]
        fcm_sb = [e(nc.sbuf_tensor(f"fcm{s}", [128, 512], dt.bfloat16)) for s in range(2)]
        h1_sb = [e(nc.sbuf_tensor(f"h1{s}", [128, 512], dt.bfloat16)) for s in range(2)]
        h2_sb = [e(nc.sbuf_tensor(f"h2{s}", [128, 512], dt.bfloat16)) for s in range(2)]
        y_sb = [e(nc.sbuf_tensor(f"ysb{s}", [128, 512 * (NSUB // 2)], dt.float32)) for s in range(2)]
        w1t_sb = e(nc.sbuf_tensor("w1ts", [128, 128], dt.bfloat16))
        w2t_sb = e(nc.sbuf_tensor("w2ts", [128, 128], dt.bfloat16))
        w3t_sb = e(nc.sbuf_tensor("w3ts", [128, 1], dt.bfloat16))
        b2_sb = e(nc.sbuf_tensor("b2s", [128, 1], dt.float32))
        ident_sb = e(nc.sbuf_tensor("idents", [128, 128], dt.bfloat16))
        # PSUM
        fT_ps = [e(nc.psum_tensor(f"ft{s}", [128, 512], dt.bfloat16)) for s in range(2)]
        v1_ps = [e(nc.psum_tensor(f"v1{s}", [128, 512], dt.float32)) for s in range(2)]
        v2_ps = [e(nc.psum_tensor(f"v2{s}", [128, 512], dt.float32)) for s in range(2)]
        yb_ps = [e(nc.psum_tensor(f"yb{s}", [128, 512], dt.float32)) for s in range(2)]

        with nc.Block() as block:
            sem = lambda n: st.enter_context(nc.semaphore(n))
            init_sem = sem("init_sem")
            g_s = [[sem(f"g{s}_{p}") for p in range(3)] for s in range(2)]
            f_sem = sem("f_sem"); a_sem = sem("a_sem")
            pe_ft = sem("pe_ft"); pe_v1 = sem("pe_v1"); pe_v2 = sem("pe_v2"); pe_yb = sem("pe_yb")
            ac_fcm = sem("ac_fcm"); ac_h1 = sem("ac_h1"); ac_h2 = sem("ac_h2"); ac_y = sem("ac_y")
            ild = [sem("ild0"), sem("ild1")]; wld = [sem("wld0"), sem("wld1")]
            ald = [sem("ald0"), sem("ald1")]; xld = [sem("xld0"), sem("xld1")]
            out_s = [sem("out0"), sem("out1")]
            NINIT = 5 * 16

            @block.sync
            def _(sync):
                sync.dma_start(out=w1t_sb[:, :], in_=w1t_d[:, :]).then_inc(init_sem, 16)
                sync.dma_start(out=w2t_sb[:, :], in_=w2t_d[:, :]).then_inc(init_sem, 16)
                sync.dma_start(out=w3t_sb[:, :], in_=w3t_d[:, :]).then_inc(init_sem, 16)
                sync.dma_start(out=b2_sb[:, :], in_=b2_d[:, :]).then_inc(init_sem, 16)
                sync.dma_start(out=ident_sb[:, :], in_=ident_d[:, :]).then_inc(init_sem, 16)
                for i in range(NTILES):
                    sl = i % 2
                    if i >= 2:
                        # WAR: slot consumers of tile i-2 must be done.
                        for p in range(3):   # idx read by gathers of tile i-2
                            sync.wait_ge(g_s[i % 2][p], 16 * ((i - 2) // 2 + 1))
                        sync.wait_ge(f_sem, i - 1)              # wts/xpt read by DVE
                        sync.wait_ge(a_sem, i - 1)              # args read by ACT
                    sync.dma_start(out=idx_sb[sl][:, :], in_=idx_d[i]).then_inc(ild[sl], 16)
                    sync.dma_start(out=wts_sb[sl][:, :], in_=wts_d[i]).then_inc(wld[sl], 16)
                    sync.dma_start(out=args_sb[sl][:, :], in_=args_d[i]).then_inc(ald[sl], 16)
                    sync.dma_start(out=xpt_sb[sl][:, :], in_=xpt_d[i]).then_inc(xld[sl], 16)
                    if i >= 2:
                        io = i - 2
                        for g in range(4):
                            sync.wait_ge(ac_y, 4 * io + g + 1)
                            sync.dma_start(
                                out=y_d[io, 2 * g:2 * (g + 1), :],
                                in_=y_sb[io % 2][0:64:32, 512 * g:512 * (g + 1)],
                            ).then_inc(out_s[io % 2], 16)
                for io in (NTILES - 2, NTILES - 1):
                    for g in range(4):
                        sync.wait_ge(ac_y, 4 * io + g + 1)
                        sync.dma_start(
                            out=y_d[io, 2 * g:2 * (g + 1), :],
                            in_=y_sb[io % 2][0:64:32, 512 * g:512 * (g + 1)],
                        ).then_inc(out_s[io % 2], 16)
                sync.wait_ge(out_s[0], 16 * 4 * ((NTILES + 1) // 2))
                sync.wait_ge(out_s[1], 16 * 4 * (NTILES // 2))

            @block.gpsimd
            def _(gpsimd):
                nogather = os.environ.get("K_NOGATHER") == "1"
                nidx_reg = gpsimd.alloc_register("nidx")
                gpsimd.reg_mov(nidx_reg, TT)
                for i in range(NTILES):
                    sl = i % 2
                    gpsimd.wait_ge(ild[sl], 16 * (i // 2 + 1))   # idx loaded
                    if i >= 2:
                        gpsimd.wait_ge(f_sem, i - 1)             # G slot free
                    for p in range(3):
                        if nogather:
                            gpsimd.dma_start(
                                out=G_sb[sl][p][:, :],
                                in_=tabs[p][0:ST, :].rearrange(
                                    "r v -> (r v)").unsqueeze(0).broadcast_to(
                                    (128, ST * 128)),
                            ).then_inc(g_s[sl][p], 16)
                            continue
                        gpsimd.dma_gather(
                            G_sb[sl][p][:, :].rearrange("q (s v) -> q s v", v=128),
                            tabs[p][:, :],
                            idx_sb[sl][:, p * IDXF:(p + 1) * IDXF],
                            TT,
                            nidx_reg,
                            128,
                            single_packet=False,
                        ).then_inc(g_s[sl][p], 16)

            @block.vector
            def _(vector):
                for s in range(2):
                    vector.memset(yb_ps[s][:, :], 0.0)
                vector.drain()
                # init constant feature rows: col 123 = 1.0 (bias row), 124..127 = 0
                for s in range(2):
                    fr = f_sb[s][:, :].rearrange("q (s v) -> q s v", v=128)
                    vector.memset(fr[:, :, 123:124], 1.0)
                    vector.memset(fr[:, :, 124:128], 0.0)
                for i in range(NTILES):
                    sl = i % 2
                    vector.wait_ge(wld[sl], 16 * (i // 2 + 1))   # wts loaded
                    vector.wait_ge(xld[sl], 16 * (i // 2 + 1))   # xpt loaded
                    if i >= 2:
                        vector.wait_ge(pe_ft, 8 * (i - 1))       # f slot free
                    fr = f_sb[sl][:, :].rearrange("q (s v) -> q s v", v=128)
                    wr = wts_sb[sl][:, :].rearrange("q (s w) -> q s w", w=12)
                    m = [m_sb[j][:, :].rearrange("q (s v) -> q s v", v=32)
                         for j in range(9)]
                    gr = [G_sb[sl][p][:, :].rearrange("q (s v) -> q s v", v=128)
                          for p in range(3)]
                    for p in range(3):
                        # start as soon as THIS plane's gather has landed
                        vector.wait_ge(g_s[sl][p], 16 * (i // 2 + 1))
                        for t in range(3):
                            w = wr[:, :, 3 * p + t:3 * p + t + 1].broadcast_to(
                                (128, ST, 32))
                            vector.tensor_tensor(
                                m[3 * p + t], gr[p][:, :, 32 * (t + 1):32 * (t + 2)],
                                w, Alu.mult)
                    vector.drain()
                    for p in range(3):
                        vector.tensor_tensor(m[3 * p], m[3 * p], m[3 * p + 1], Alu.add)
                    vector.drain()
                    for p in range(3):
                        vector.tensor_tensor(m[3 * p], m[3 * p], m[3 * p + 2], Alu.add)
                    vector.drain()
                    for p in range(3):
                        vector.tensor_tensor(
                            fr[:, :, 32 * p:32 * (p + 1)], m[3 * p],
                            gr[p][:, :, 0:32], Alu.add)
                    xr = xpt_sb[sl][:, :].rearrange("q (s v) -> q s v", v=3)
                    vector.drain()
                    vector.tensor_copy(fr[:, :, 96:99], xr).then_inc(f_sem, 1)

            @block.scalar
            def _(scalar):
                for i in range(NTILES):
                    sl = i % 2
                    scalar.wait_ge(ald[sl], 16 * (i // 2 + 1))   # args loaded
                    if i >= 2:
                        scalar.wait_ge(pe_ft, 8 * (i - 1))       # f slot free
                    fr = f_sb[sl][:, :].rearrange("q (s v) -> q s v", v=128)
                    ar = args_sb[sl][:, :].rearrange("q (s v) -> q s v", v=24)
                    scalar.activation(fr[:, :, 99:123], ar, Act.Sin).then_inc(a_sem, 1)
                    for k in range(NSUB + 3):
                        if k < NSUB:
                            kg = 8 * i + k
                            scalar.wait_ge(pe_ft, kg + 1)
                            if kg >= 2:
                                scalar.wait_ge(pe_v1, kg - 1)    # fcm_sb slot free
                            scalar.activation(
                                fcm_sb[k % 2][:, :], fT_ps[k % 2][:, :], Act.Copy
                            ).then_inc(ac_fcm, 1)
                        j = k - 1
                        if 0 <= j < NSUB:
                            jg = 8 * i + j
                            scalar.wait_ge(pe_v1, jg + 1)
                            if jg >= 2:
                                scalar.wait_ge(pe_v2, jg - 1)    # h1_sb slot free
                            scalar.activation(
                                h1_sb[j % 2][:, :], v1_ps[j % 2][:, :], Act.Relu
                            ).then_inc(ac_h1, 1)
                        j = k - 2
                        if 0 <= j < NSUB:
                            jg = 8 * i + j
                            scalar.wait_ge(pe_v2, jg + 1)
                            if jg >= 2:
                                scalar.wait_ge(pe_yb, jg - 1)    # h2_sb slot free
                            scalar.activation(
                                h2_sb[j % 2][:, :], v2_ps[j % 2][:, :], Act.Relu,
                                bias=b2_sb[:, 0:1],
                            ).then_inc(ac_h2, 1)
                        j = k - 3
                        if 0 <= j < NSUB and j % 2 == 1:
                            g = j // 2
                            gg = 4 * i + g
                            scalar.wait_ge(pe_yb, 8 * i + j + 1)
                            if i >= 2 and g == 0:
                                scalar.wait_ge(out_s[i % 2], 16 * 4 * ((i - 2) // 2 + 1))
                            scalar.activation(
                                y_sb[i % 2][0:64, 512 * g:512 * (g + 1)],
                                yb_ps[g % 2][0:64, :],
                                Act.Copy,
                            ).then_inc(ac_y, 1)

            @block.tensor
            def _(tensor):
                tensor.wait_ge(init_sem, NINIT)
                for i in range(NTILES):
                    sl = i % 2
                    fr = f_sb[sl][:, :].rearrange("q (s v) -> q s v", v=128)
                    for k in range(NSUB + 3):
                        if k < NSUB:
                            kg = 8 * i + k
                            if k == 0:
                                tensor.wait_ge(f_sem, i + 1)
                                tensor.wait_ge(a_sem, i + 1)
                            if kg >= 2:
                                tensor.wait_ge(ac_fcm, kg - 1)   # fT bank free
                            for b in range(4):
                                ins = tensor.transpose(
                                    fT_ps[k % 2][:, 128 * b:128 * (b + 1)],
                                    fr[:, 4 * k + b, :],
                                    ident_sb[:, :],
                                )
                            ins.then_inc(pe_ft, 1)
                        j = k - 1
                        if 0 <= j < NSUB:
                            jg = 8 * i + j
                            tensor.wait_ge(ac_fcm, jg + 1)
                            if jg >= 2:
                                tensor.wait_ge(ac_h1, jg - 1)    # v1 bank free
                            tensor.matmul(
                                v1_ps[j % 2][:, :], w1t_sb[:, :], fcm_sb[j % 2][:, :]
                            ).then_inc(pe_v1, 1)
                        j = k - 2
                        if 0 <= j < NSUB:
                            jg = 8 * i + j
                            tensor.wait_ge(ac_h1, jg + 1)
                            if jg >= 2:
                                tensor.wait_ge(ac_h2, jg - 1)    # v2 bank free
                            tensor.matmul(
                                v2_ps[j % 2][:, :], w2t_sb[:, :], h1_sb[j % 2][:, :]
                            ).then_inc(pe_v2, 1)
                        j = k - 3
                        if 0 <= j < NSUB:
                            jg = 8 * i + j
                            g = j // 2
                            gg = 4 * i + g
                            tensor.wait_ge(ac_h2, jg + 1)
                            if gg >= 2 and j % 2 == 0:
                                tensor.wait_ge(ac_y, gg - 1)     # yb bank free
                            tensor.matmul(
                                yb_ps[g % 2][32 * (j % 2):32 * (j % 2) + 1, :],
                                w3t_sb[:, :],
                                h2_sb[j % 2][:, :],
                            ).then_inc(pe_yb, 1)

    nc.compile()
    return nc


def _host_prep(coordinates, plane0, plane1, plane2, W1, b1, W2, b2, W3, b3):
    """Build all device inputs. Returns (shared, per_core_list, b3)."""
    f32 = np.float32
    pts = np.ascontiguousarray(coordinates.reshape(NPTS, 3).astype(f32))

    # --- tables ----------------------------------------------------------
    tabs = []
    for pl in (plane0, plane1, plane2):
        q = np.asarray(pl, dtype=f32)[:, 127:256, 127:256]      # [32,129,129]
        g00 = q[:, :128, :128]
        g01 = q[:, :128, 1:129]
        g10 = q[:, 1:129, :128]
        g11 = q[:, 1:129, 1:129]
        d = np.stack([g00, g01 - g00, g10 - g00, g11 - g01 - g10 + g00], axis=0)
        # [term, c, ly, lx] -> [ly, lx, term, c] -> [16384, 128]
        tab = np.transpose(d, (2, 3, 0, 1)).reshape(NROWS, 128).astype(BF16)
        tabs.append(np.ascontiguousarray(tab))

    # --- per-point quantities -------------------------------------------
    fx = (pts + f32(1.0)) * f32(0.5) * f32(255.0)               # [NPTS,3]
    x0 = np.floor(fx)
    fr = (fx - x0).astype(f32)                                  # fractional parts
    cell = (x0.astype(np.int32) - 127)                          # [NPTS,3] in [0,127]

    idx_all = np.empty((NPTS, 3), np.int16)
    wts_all = np.zeros((NPTS, 12), f32)
    for p, (ua, va) in enumerate(PLANE_DIMS):
        idx_all[:, p] = (cell[:, va] * NCELL + cell[:, ua]).astype(np.int16)
        wts_all[:, 3 * p + 0] = fr[:, ua]
        wts_all[:, 3 * p + 1] = fr[:, va]
        wts_all[:, 3 * p + 2] = fr[:, ua] * fr[:, va]
    wts_all = wts_all.astype(BF16)

    freqs = (2.0 ** np.linspace(0.0, MULTIRES - 1.0, MULTIRES)).astype(f32)
    args_all = np.empty((NPTS, 24), f32)
    for i, f in enumerate(freqs):
        args_all[:, 6 * i:6 * i + 3] = pts * f
        args_all[:, 6 * i + 3:6 * i + 6] = pts * f + f32(np.pi / 2)
    # ACT Sin domain is [-pi, pi]: exact periodic range reduction (float64).
    a64 = args_all.astype(np.float64)
    a64 = a64 - 2 * np.pi * np.round(a64 / (2 * np.pi))
    args_all = np.clip(a64, -np.pi, np.pi).astype(np.float16)
    xpt_all = pts.astype(BF16)

    # --- weights ---------------------------------------------------------
    w1t = np.zeros((128, 128), f32)
    w1t[:123, :] = np.asarray(W1, f32).T                        # [123,128]
    w1t[123, :] = np.asarray(b1, f32)
    w2t = np.asarray(W2, f32).T.astype(BF16)                    # [128,128]
    w3t = np.asarray(W3, f32).T.astype(BF16)                    # [128,1]
    b2c = np.ascontiguousarray(np.asarray(b2, f32).reshape(128, 1))
    ident = np.eye(128, dtype=BF16)
    shared = dict(
        tab0=tabs[0], tab1=tabs[1], tab2=tabs[2],
        w1t=w1t.astype(BF16), w2t=w2t, w3t=w3t, b2c=b2c, ident=ident,
    )

    def tile_pm(a, core):
        """[TCORE, M] slice of a per-point array -> [NTILES, 128, ST*M],
        point j=128*s+p of tile i at [i, p, s, :]."""
        m = a.shape[1]
        v = a[core * TCORE:(core + 1) * TCORE].reshape(NTILES, ST, 128, m)
        return np.ascontiguousarray(
            v.transpose(0, 2, 1, 3).reshape(NTILES, 128, ST * m)
        )

    per_core = []
    for core in range(NCORES):
        idx_c = idx_all[core * TCORE:(core + 1) * TCORE]        # [TCORE,3]
        # wrapped layout per tile/plane: [16, IDXF], idx j at [j%16, j//16],
        # then replicated x8 down partitions.
        iv = idx_c.reshape(NTILES, TT, 3).transpose(0, 2, 1)    # [NT,3,TT]
        iw = iv.reshape(NTILES, 3, IDXF, 16).transpose(0, 1, 3, 2)  # [NT,3,16,IDXF]
        iw = np.broadcast_to(iw[:, None], (NTILES, 8, 3, 16, IDXF))
        iw = iw.transpose(0, 2, 1, 3, 4).reshape(NTILES, 3, 128, IDXF)
        iw = iw.transpose(0, 2, 1, 3).reshape(NTILES, 128, 3 * IDXF)
        per_core.append(dict(
            idx=np.ascontiguousarray(iw),
            wts=tile_pm(wts_all, core),
            args=tile_pm(args_all, core),
            xpt=tile_pm(xpt_all, core),
        ))
    return shared, per_core


_NC_CACHE = {}


def _get_nc():
    if "nc" not in _NC_CACHE:
        _NC_CACHE["nc"] = build_nc()
    return _NC_CACHE["nc"]


def kernel(coordinates, plane0, plane1, plane2, W1, b1, W2, b2, W3, b3):
    args = [np.asarray(a) for a in
            (coordinates, plane0, plane1, plane2, W1, b1, W2, b2, W3, b3)]
    shared, per_core = _host_prep(*args)[:2]
    b3 = args[-1]
    nc = _get_nc()
    in_maps = [{**shared, **per_core[c]} for c in range(NCORES)]
    res = run_bass_kernel_spmd(nc, in_maps, list(range(NCORES)))
    ys = [np.asarray(res.results[c]["y"], np.float32).reshape(TCORE)
          for c in range(NCORES)]
    y = np.concatenate(ys) + np.float32(np.asarray(b3, np.float32).reshape(()))
    return y.reshape(B, N, 1).astype(np.float32)


# revision 27
# speedup vs baseline: 1.2548x; 1.0001x over previous
"""Trainium2 Bass kernel for CartesianPlaneNonSirenEmbeddingNetwork.

Tri-plane bilinear feature sampling + positional encoding + 3-layer MLP,
data-parallel over 8 NeuronCores (points sharded, planes/weights replicated).

Device strategy (per core, 131072 points):
  - Host packs each plane's used quadrant into a "quad-diff" gather table:
    one 256 B row per grid cell = [D0|D1|D2|D3] x 32 ch (bf16), so that
    bilinear = D0 + wx*D1 + wy*D2 + wx*wy*D3 (one dma_gather per point/plane).
  - GPSIMD dma_gather fetches rows point-major: G[128, ST, 128].
  - DVE does the 3-term interpolation with host-shipped per-point weights
    (broadcast-AP multiplies), writing features point-major f[128, ST, 128].
  - ACT evaluates sin() on host-shipped posenc args (f16) into f.
  - PE transposes f to channel-major and runs the 123->128->128->1 MLP with
    stationary weights; biases fold in via a constant-1 feature row (b1) and
    ACT bias on the relu copy (b2); b3 is added on host.
"""

import os
import numpy as np
import ml_dtypes

import concourse.bass as bass
import concourse.bacc as bacc
import concourse.mybir as mybir
from concourse import library_config
from concourse.bass_utils import run_bass_kernel_spmd

BF16 = ml_dtypes.bfloat16

# Problem shapes (hardcoded).
C, H, W = 32, 256, 256
MULTIRES = 4
B, N = 4, 262144
NPTS = B * N
NCORES = 8
TCORE = NPTS // NCORES          # 131072 points per core

# Tiling.
ST = 32                         # 128-point blocks per tile
TT = 128 * ST                   # 4096 points per tile
NTILES = TCORE // TT            # 32
IDXF = TT // 16                 # 256  (wrapped idx free dim per plane)
NSUB = ST // 4                  # 8    (512-point sub-chunks per tile)

NCELL = 128                     # used cells per axis (coords in [0,1))
NROWS = NCELL * NCELL           # 16384 table rows per plane

PLANE_DIMS = [(0, 1), (1, 2), (0, 2)]   # (u, v) coordinate dims per plane

dt = mybir.dt
Alu = mybir.AluOpType
Act = mybir.ActivationFunctionType


def build_nc():
    nc = bacc.Bacc()

    tabs = [
        nc.declare_dram_parameter(f"tab{p}", [NROWS, 128], dt.bfloat16, False)
        for p in range(3)
    ]
    idx_d = nc.declare_dram_parameter("idx", [NTILES, 128, 3 * IDXF], dt.int16, False)
    wts_d = nc.declare_dram_parameter("wts", [NTILES, 128, ST * 12], dt.bfloat16, False)
    args_d = nc.declare_dram_parameter("args", [NTILES, 128, ST * 24], dt.float16, False)
    xpt_d = nc.declare_dram_parameter("xpt", [NTILES, 128, ST * 3], dt.bfloat16, False)
    w1t_d = nc.declare_dram_parameter("w1t", [128, 128], dt.bfloat16, False)
    w2t_d = nc.declare_dram_parameter("w2t", [128, 128], dt.bfloat16, False)
    w3t_d = nc.declare_dram_parameter("w3t", [128, 1], dt.bfloat16, False)
    b2_d = nc.declare_dram_parameter("b2c", [128, 1], dt.float32, False)
    ident_d = nc.declare_dram_parameter("ident", [128, 128], dt.bfloat16, False)
    y_d = nc.declare_dram_parameter("y", [NTILES, 8, 512], dt.float32, True)

    from contextlib import ExitStack

    with ExitStack() as st:
        e = st.enter_context
        # SBUF
        G_sb = [[e(nc.sbuf_tensor(f"g{s}_{p}", [128, ST * 128], dt.bfloat16))
                 for p in range(3)] for s in range(2)]
        idx_sb = [e(nc.sbuf_tensor(f"idx{s}", [128, 3 * IDXF], dt.int16)) for s in range(2)]
        wts_sb = [e(nc.sbuf_tensor(f"wts{s}", [128, ST * 12], dt.bfloat16)) for s in range(2)]
        args_sb = [e(nc.sbuf_tensor(f"args{s}", [128, ST * 24], dt.float16)) for s in range(2)]
        xpt_sb = [e(nc.sbuf_tensor(f"xpt{s}", [128, ST * 3], dt.bfloat16)) for s in range(2)]
        f_sb = [e(nc.sbuf_tensor(f"f{s}", [128, ST * 128], dt.bfloat16)) for s in range(2)]
        m_sb = [e(nc.sbuf_tensor(f"m{j}", [128, ST * 96], dt.bfloat16))
                for j in range(3)]
        fcm_sb = [e(nc.sbuf_tensor(f"fcm{s}", [128, 512], dt.bfloat16)) for s in range(2)]
        h1_sb = [e(nc.sbuf_tensor(f"h1{s}", [128, 512], dt.bfloat16)) for s in range(2)]
        h2_sb = [e(nc.sbuf_tensor(f"h2{s}", [128, 512], dt.bfloat16)) for s in range(2)]
        y_sb = [e(nc.sbuf_tensor(f"ysb{s}", [128, 512 * (NSUB // 2)], dt.float32)) for s in range(2)]
        w1t_sb = e(nc.sbuf_tensor("w1ts", [128, 128], dt.bfloat16))
        w2t_sb = e(nc.sbuf_tensor("w2ts", [128, 128], dt.bfloat16))
        w3t_sb = e(nc.sbuf_tensor("w3ts", [128, 1], dt.bfloat16))
        b2_sb = e(nc.sbuf_tensor("b2s", [128, 1], dt.float32))
        ident_sb = e(nc.sbuf_tensor("idents", [128, 128], dt.bfloat16))
        # PSUM
        fT_ps = [e(nc.psum_tensor(f"ft{s}", [128, 512], dt.bfloat16)) for s in range(2)]
        v1_ps = [e(nc.psum_tensor(f"v1{s}", [128, 512], dt.float32)) for s in range(2)]
        v2_ps = [e(nc.psum_tensor(f"v2{s}", [128, 512], dt.float32)) for s in range(2)]
        yb_ps = [e(nc.psum_tensor(f"yb{s}", [128, 512], dt.float32)) for s in range(2)]

        with nc.Block() as block:
            sem = lambda n: st.enter_context(nc.semaphore(n))
            init_sem = sem("init_sem")
            g_s = [[sem(f"g{s}_{p}") for p in range(3)] for s in range(2)]
            f_sem = sem("f_sem"); a_sem = sem("a_sem")
            pe_ft = sem("pe_ft"); pe_v1 = sem("pe_v1"); pe_v2 = sem("pe_v2"); pe_yb = sem("pe_yb")
            ac_fcm = sem("ac_fcm"); ac_h1 = sem("ac_h1"); ac_h2 = sem("ac_h2"); ac_y = sem("ac_y")
            ild = [sem("ild0"), sem("ild1")]; wld = [sem("wld0"), sem("wld1")]
            ald = [sem("ald0"), sem("ald1")]; xld = [sem("xld0"), sem("xld1")]
            out_s = [sem("out0"), sem("out1")]
            NINIT = 5 * 16

            @block.sync
            def _(sync):
                sync.dma_start(out=w1t_sb[:, :], in_=w1t_d[:, :]).then_inc(init_sem, 16)
                sync.dma_start(out=w2t_sb[:, :], in_=w2t_d[:, :]).then_inc(init_sem, 16)
                sync.dma_start(out=w3t_sb[:, :], in_=w3t_d[:, :]).then_inc(init_sem, 16)
                sync.dma_start(out=b2_sb[:, :], in_=b2_d[:, :]).then_inc(init_sem, 16)
                sync.dma_start(out=ident_sb[:, :], in_=ident_d[:, :]).then_inc(init_sem, 16)
                for i in range(NTILES):
                    sl = i % 2
                    if i >= 2:
                        # WAR: slot consumers of tile i-2 must be done.
                        for p in range(3):   # idx read by gathers of tile i-2
                            sync.wait_ge(g_s[i % 2][p], 16 * ((i - 2) // 2 + 1))
                        sync.wait_ge(f_sem, i - 1)              # wts/xpt read by DVE
                        sync.wait_ge(a_sem, i - 1)              # args read by ACT
                    sync.dma_start(out=idx_sb[sl][:, :], in_=idx_d[i]).then_inc(ild[sl], 16)
                    sync.dma_start(out=wts_sb[sl][:, :], in_=wts_d[i]).then_inc(wld[sl], 16)
                    sync.dma_start(out=args_sb[sl][:, :], in_=args_d[i]).then_inc(ald[sl], 16)
                    sync.dma_start(out=xpt_sb[sl][:, :], in_=xpt_d[i]).then_inc(xld[sl], 16)
                    if i >= 2:
                        io = i - 2
                        for g in range(4):
                            sync.wait_ge(ac_y, 4 * io + g + 1)
                            sync.dma_start(
                                out=y_d[io, 2 * g:2 * (g + 1), :],
                                in_=y_sb[io % 2][0:64:32, 512 * g:512 * (g + 1)],
                            ).then_inc(out_s[io % 2], 16)
                for io in (NTILES - 2, NTILES - 1):
                    for g in range(4):
                        sync.wait_ge(ac_y, 4 * io + g + 1)
                        sync.dma_start(
                            out=y_d[io, 2 * g:2 * (g + 1), :],
                            in_=y_sb[io % 2][0:64:32, 512 * g:512 * (g + 1)],
                        ).then_inc(out_s[io % 2], 16)
                sync.wait_ge(out_s[0], 16 * 4 * ((NTILES + 1) // 2))
                sync.wait_ge(out_s[1], 16 * 4 * (NTILES // 2))

            @block.gpsimd
            def _(gpsimd):
                nogather = os.environ.get("K_NOGATHER") == "1"
                nidx_reg = gpsimd.alloc_register("nidx")
                gpsimd.reg_mov(nidx_reg, TT)
                for i in range(NTILES):
                    sl = i % 2
                    gpsimd.wait_ge(ild[sl], 16 * (i // 2 + 1))   # idx loaded
                    if i >= 2:
                        gpsimd.wait_ge(f_sem, i - 1)             # G slot free
                    for p in range(3):
                        if nogather:
                            gpsimd.dma_start(
                                out=G_sb[sl][p][:, :],
                                in_=tabs[p][0:ST, :].rearrange(
                                    "r v -> (r v)").unsqueeze(0).broadcast_to(
                                    (128, ST * 128)),
                            ).then_inc(g_s[sl][p], 16)
                            continue
                        gpsimd.dma_gather(
                            G_sb[sl][p][:, :].rearrange("q (s v) -> q s v", v=128),
                            tabs[p][:, :],
                            idx_sb[sl][:, p * IDXF:(p + 1) * IDXF],
                            TT,
                            nidx_reg,
                            128,
                            single_packet=False,
                        ).then_inc(g_s[sl][p], 16)

            @block.vector
            def _(vector):
                for s in range(2):
                    vector.memset(yb_ps[s][:, :], 0.0)
                vector.drain()
                # init constant feature rows: col 123 = 1.0 (bias row), 124..127 = 0
                for s in range(2):
                    fr = f_sb[s][:, :].rearrange("q (s v) -> q s v", v=128)
                    vector.memset(fr[:, :, 123:124], 1.0)
                    vector.memset(fr[:, :, 124:128], 0.0)
                for i in range(NTILES):
                    sl = i % 2
                    vector.wait_ge(wld[sl], 16 * (i // 2 + 1))   # wts loaded
                    vector.wait_ge(xld[sl], 16 * (i // 2 + 1))   # xpt loaded
                    if i >= 2:
                        vector.wait_ge(pe_ft, 8 * (i - 1))       # f slot free
                    fr = f_sb[sl][:, :].rearrange("q (s v) -> q s v", v=128)
                    wr = wts_sb[sl][:, :].rearrange("q (s w) -> q s w", w=12)
                    gr = [G_sb[sl][p][:, :].rearrange("q (s v) -> q s v", v=128)
                          for p in range(3)]
                    for p in range(3):
                        # start as soon as THIS plane's gather has landed
                        vector.wait_ge(g_s[sl][p], 16 * (i // 2 + 1))
                        # one fused multiply per plane: [D1|D2|D3] * [wx|wy|wxy]
                        w3 = wr[:, :, 3 * p:3 * p + 3].unsqueeze(3).broadcast_to(
                            (128, ST, 3, 32))
                        g3 = gr[p][:, :, 32:128].rearrange(
                            "q s (t v) -> q s t v", v=32)
                        mt = m_sb[p][:, :].rearrange(
                            "q (s t v) -> q s t v", t=3, v=32)
                        vector.tensor_tensor(mt, g3, w3, Alu.mult)
                    vector.drain()
                    mts = [m_sb[p][:, :].rearrange("q (s t v) -> q s t v", t=3, v=32)
                           for p in range(3)]
                    for p in range(3):
                        vector.tensor_tensor(
                            mts[p][:, :, 0, :], mts[p][:, :, 0, :],
                            mts[p][:, :, 1, :], Alu.add)
                    vector.drain()
                    for p in range(3):
                        vector.tensor_tensor(
                            mts[p][:, :, 0, :], mts[p][:, :, 0, :],
                            mts[p][:, :, 2, :], Alu.add)
                    vector.drain()
                    for p in range(3):
                        vector.tensor_tensor(
                            fr[:, :, 32 * p:32 * (p + 1)], mts[p][:, :, 0, :],
                            gr[p][:, :, 0:32], Alu.add)
                    xr = xpt_sb[sl][:, :].rearrange("q (s v) -> q s v", v=3)
                    vector.drain()
                    vector.tensor_copy(fr[:, :, 96:99], xr).then_inc(f_sem, 1)

            @block.scalar
            def _(scalar):
                for i in range(NTILES):
                    sl = i % 2
                    scalar.wait_ge(ald[sl], 16 * (i // 2 + 1))   # args loaded
                    if i >= 2:
                        scalar.wait_ge(pe_ft, 8 * (i - 1))       # f slot free
                    fr = f_sb[sl][:, :].rearrange("q (s v) -> q s v", v=128)
                    ar = args_sb[sl][:, :].rearrange("q (s v) -> q s v", v=24)
                    scalar.activation(fr[:, :, 99:123], ar, Act.Sin).then_inc(a_sem, 1)
                    for k in range(NSUB + 3):
                        if k < NSUB:
                            kg = 8 * i + k
                            scalar.wait_ge(pe_ft, kg + 1)
                            if kg >= 2:
                                scalar.wait_ge(pe_v1, kg - 1)    # fcm_sb slot free
                            scalar.activation(
                                fcm_sb[k % 2][:, :], fT_ps[k % 2][:, :], Act.Copy
                            ).then_inc(ac_fcm, 1)
                        j = k - 1
                        if 0 <= j < NSUB:
                            jg = 8 * i + j
                            scalar.wait_ge(pe_v1, jg + 1)
                            if jg >= 2:
                                scalar.wait_ge(pe_v2, jg - 1)    # h1_sb slot free
                            scalar.activation(
                                h1_sb[j % 2][:, :], v1_ps[j % 2][:, :], Act.Relu
                            ).then_inc(ac_h1, 1)
                        j = k - 2
                        if 0 <= j < NSUB:
                            jg = 8 * i + j
                            scalar.wait_ge(pe_v2, jg + 1)
                            if jg >= 2:
                                scalar.wait_ge(pe_yb, jg - 1)    # h2_sb slot free
                            scalar.activation(
                                h2_sb[j % 2][:, :], v2_ps[j % 2][:, :], Act.Relu,
                                bias=b2_sb[:, 0:1],
                            ).then_inc(ac_h2, 1)
                        j = k - 3
                        if 0 <= j < NSUB and j % 2 == 1:
                            g = j // 2
                            gg = 4 * i + g
                            scalar.wait_ge(pe_yb, 8 * i + j + 1)
                            if i >= 2 and g == 0:
                                scalar.wait_ge(out_s[i % 2], 16 * 4 * ((i - 2) // 2 + 1))
                            scalar.activation(
                                y_sb[i % 2][0:64, 512 * g:512 * (g + 1)],
                                yb_ps[g % 2][0:64, :],
                                Act.Copy,
                            ).then_inc(ac_y, 1)

            @block.tensor
            def _(tensor):
                tensor.wait_ge(init_sem, NINIT)
                for i in range(NTILES):
                    sl = i % 2
                    fr = f_sb[sl][:, :].rearrange("q (s v) -> q s v", v=128)
                    for k in range(NSUB + 3):
                        if k < NSUB:
                            kg = 8 * i + k
                            if k == 0:
                                tensor.wait_ge(f_sem, i + 1)
                                tensor.wait_ge(a_sem, i + 1)
                            if kg >= 2:
                                tensor.wait_ge(ac_fcm, kg - 1)   # fT bank free
                            for b in range(4):
                                ins = tensor.transpose(
                                    fT_ps[k % 2][:, 128 * b:128 * (b + 1)],
                                    fr[:, 4 * k + b, :],
                                    ident_sb[:, :],
                                )
                            ins.then_inc(pe_ft, 1)
                        j = k - 1
                        if 0 <= j < NSUB:
                            jg = 8 * i + j
                            tensor.wait_ge(ac_fcm, jg + 1)
                            if jg >= 2:
                                tensor.wait_ge(ac_h1, jg - 1)    # v1 bank free
                            tensor.matmul(
                                v1_ps[j % 2][:, :], w1t_sb[:, :], fcm_sb[j % 2][:, :]
                            ).then_inc(pe_v1, 1)
                        j = k - 2
                        if 0 <= j < NSUB:
                            jg = 8 * i + j
                            tensor.wait_ge(ac_h1, jg + 1)
                            if jg >= 2:
                                tensor.wait_ge(ac_h2, jg - 1)    # v2 bank free
                            tensor.matmul(
                                v2_ps[j % 2][:, :], w2t_sb[:, :], h1_sb[j % 2][:, :]
                            ).then_inc(pe_v2, 1)
                        j = k - 3
                        if 0 <= j < NSUB:
                            jg = 8 * i + j
                            g = j // 2
                            gg = 4 * i + g
                            tensor.wait_ge(ac_h2, jg + 1)
                            if gg >= 2 and j % 2 == 0:
                                tensor.wait_ge(ac_y, gg - 1)     # yb bank free
                            tensor.matmul(
                                yb_ps[g % 2][32 * (j % 2):32 * (j % 2) + 1, :],
                                w3t_sb[:, :],
                                h2_sb[j % 2][:, :],
                            ).then_inc(pe_yb, 1)

    nc.compile()
    return nc


def _host_prep(coordinates, plane0, plane1, plane2, W1, b1, W2, b2, W3, b3):
    """Build all device inputs. Returns (shared, per_core_list, b3)."""
    f32 = np.float32
    pts = np.ascontiguousarray(coordinates.reshape(NPTS, 3).astype(f32))

    # --- tables ----------------------------------------------------------
    tabs = []
    for pl in (plane0, plane1, plane2):
        q = np.asarray(pl, dtype=f32)[:, 127:256, 127:256]      # [32,129,129]
        g00 = q[:, :128, :128]
        g01 = q[:, :128, 1:129]
        g10 = q[:, 1:129, :128]
        g11 = q[:, 1:129, 1:129]
        d = np.stack([g00, g01 - g00, g10 - g00, g11 - g01 - g10 + g00], axis=0)
        # [term, c, ly, lx] -> [ly, lx, term, c] -> [16384, 128]
        tab = np.transpose(d, (2, 3, 0, 1)).reshape(NROWS, 128).astype(BF16)
        tabs.append(np.ascontiguousarray(tab))

    # --- per-point quantities -------------------------------------------
    fx = (pts + f32(1.0)) * f32(0.5) * f32(255.0)               # [NPTS,3]
    x0 = np.floor(fx)
    fr = (fx - x0).astype(f32)                                  # fractional parts
    cell = (x0.astype(np.int32) - 127)                          # [NPTS,3] in [0,127]

    idx_all = np.empty((NPTS, 3), np.int16)
    wts_all = np.zeros((NPTS, 12), f32)
    for p, (ua, va) in enumerate(PLANE_DIMS):
        idx_all[:, p] = (cell[:, va] * NCELL + cell[:, ua]).astype(np.int16)
        wts_all[:, 3 * p + 0] = fr[:, ua]
        wts_all[:, 3 * p + 1] = fr[:, va]
        wts_all[:, 3 * p + 2] = fr[:, ua] * fr[:, va]
    wts_all = wts_all.astype(BF16)

    freqs = (2.0 ** np.linspace(0.0, MULTIRES - 1.0, MULTIRES)).astype(f32)
    args_all = np.empty((NPTS, 24), f32)
    for i, f in enumerate(freqs):
        args_all[:, 6 * i:6 * i + 3] = pts * f
        args_all[:, 6 * i + 3:6 * i + 6] = pts * f + f32(np.pi / 2)
    # ACT Sin domain is [-pi, pi]: exact periodic range reduction (float64).
    a64 = args_all.astype(np.float64)
    a64 = a64 - 2 * np.pi * np.round(a64 / (2 * np.pi))
    args_all = np.clip(a64, -np.pi, np.pi).astype(np.float16)
    xpt_all = pts.astype(BF16)

    # --- weights ---------------------------------------------------------
    w1t = np.zeros((128, 128), f32)
    w1t[:123, :] = np.asarray(W1, f32).T                        # [123,128]
    w1t[123, :] = np.asarray(b1, f32)
    w2t = np.asarray(W2, f32).T.astype(BF16)                    # [128,128]
    w3t = np.asarray(W3, f32).T.astype(BF16)                    # [128,1]
    b2c = np.ascontiguousarray(np.asarray(b2, f32).reshape(128, 1))
    ident = np.eye(128, dtype=BF16)
    shared = dict(
        tab0=tabs[0], tab1=tabs[1], tab2=tabs[2],
        w1t=w1t.astype(BF16), w2t=w2t, w3t=w3t, b2c=b2c, ident=ident,
    )

    def tile_pm(a, core):
        """[TCORE, M] slice of a per-point array -> [NTILES, 128, ST*M],
        point j=128*s+p of tile i at [i, p, s, :]."""
        m = a.shape[1]
        v = a[core * TCORE:(core + 1) * TCORE].reshape(NTILES, ST, 128, m)
        return np.ascontiguousarray(
            v.transpose(0, 2, 1, 3).reshape(NTILES, 128, ST * m)
        )

    per_core = []
    for core in range(NCORES):
        idx_c = idx_all[core * TCORE:(core + 1) * TCORE]        # [TCORE,3]
        # wrapped layout per tile/plane: [16, IDXF], idx j at [j%16, j//16],
        # then replicated x8 down partitions.
        iv = idx_c.reshape(NTILES, TT, 3).transpose(0, 2, 1)    # [NT,3,TT]
        iw = iv.reshape(NTILES, 3, IDXF, 16).transpose(0, 1, 3, 2)  # [NT,3,16,IDXF]
        iw = np.broadcast_to(iw[:, None], (NTILES, 8, 3, 16, IDXF))
        iw = iw.transpose(0, 2, 1, 3, 4).reshape(NTILES, 3, 128, IDXF)
        iw = iw.transpose(0, 2, 1, 3).reshape(NTILES, 128, 3 * IDXF)
        per_core.append(dict(
            idx=np.ascontiguousarray(iw),
            wts=tile_pm(wts_all, core),
            args=tile_pm(args_all, core),
            xpt=tile_pm(xpt_all, core),
        ))
    return shared, per_core


_NC_CACHE = {}


def _get_nc():
    if "nc" not in _NC_CACHE:
        _NC_CACHE["nc"] = build_nc()
    return _NC_CACHE["nc"]


def kernel(coordinates, plane0, plane1, plane2, W1, b1, W2, b2, W3, b3):
    args = [np.asarray(a) for a in
            (coordinates, plane0, plane1, plane2, W1, b1, W2, b2, W3, b3)]
    shared, per_core = _host_prep(*args)[:2]
    b3 = args[-1]
    nc = _get_nc()
    in_maps = [{**shared, **per_core[c]} for c in range(NCORES)]
    res = run_bass_kernel_spmd(nc, in_maps, list(range(NCORES)))
    ys = [np.asarray(res.results[c]["y"], np.float32).reshape(TCORE)
          for c in range(NCORES)]
    y = np.concatenate(ys) + np.float32(np.asarray(b3, np.float32).reshape(()))
    return y.reshape(B, N, 1).astype(np.float32)
